# revision 8
# baseline (speedup 1.0000x reference)
"""Trainium2 Bass kernel for nn_BigramHash (hashed-bigram embedding + projection).

Computation (per reference):
    bigram_idx = pad_left0((idx[:, :-1] * 10007 + idx[:, 1:]) % 8192)   # [B, S]
    h = table[bigram_idx]                                               # fp16 [B, S, 48]
    out = h.astype(f32) @ proj_w.T                                      # f32 [B, S, 512]

Strategy (8-core data parallel over batch, 4 rows = 16384 tokens per core):
  - hash on DVE in int32 using (a & 8191) * 1815 + b (mod-2^13-equivalent,
    products < 2^24 so exact on any ALU path), then & 8191, cast to int16.
  - dma_gather(transpose=True) from the 256B-padded table in DRAM lands
    h^T in SBUF: partitions = d_bigram, free dim = tokens.
  - PE: per 128-token tile, lhsT = h^T slice [48, 128] (stationary),
    rhs = proj^T [48, 512] (optionally fp16 hi+lo pair accumulated in one
    PSUM group) -> out tile [128 tokens, 512] f32.
  - PSUM -> SBUF copies split DVE/ACT, then 1 MiB contiguous DMAs to DRAM.
"""

import os
import sys

sys.path.insert(0, "/opt/trn_rl_repo")

import numpy as np

N_CORES = 8
B, S = 32, 4096
BUCKETS, D_BIGRAM, DIM = 8192, 48, 512
ROWS_PER_CORE = B // N_CORES          # 4
NTOK = ROWS_PER_CORE * S              # 16384 tokens per core
ELEM = 128                            # padded table row: 128 x 2B = 256 B
P = 128

# Tunables (env-overridable for A/B benchmarking)
CHUNK = int(os.environ.get("KBH_CHUNK", "2048"))      # tokens per dma_gather
GRP = 4                                               # 128-token tiles per output DMA
HILO = os.environ.get("KBH_HILO", "0") == "1"         # hi/lo split of proj (2 matmuls)
DTYPE = os.environ.get("KBH_DTYPE", "f16")            # f16 | bf16 for h and proj
HT_BUFS = int(os.environ.get("KBH_HT_BUFS", "4"))
NQ = int(os.environ.get("KBH_NQ", "1"))               # SWDGE queues for gathers
SP = os.environ.get("KBH_SP", "0") == "1"             # single_packet gathers

_CACHE: dict = {}


def _np_dt():
    if DTYPE == "f16":
        return np.float16
    import ml_dtypes
    return ml_dtypes.bfloat16


def _build(ntok: int, s_row: int, chunk: int, debug: bool = False):
    """Build the per-core Bass module. ntok tokens, rows of s_row tokens."""
    import concourse.mybir as mybir
    import concourse.tile as tile
    from concourse import bacc

    assert ntok % chunk == 0 and chunk % 128 == 0 and s_row % 16 == 0
    cols = ntok // 16                 # wrapped idx columns
    ccols = chunk // 16               # wrapped idx columns per chunk
    nch = ntok // chunk               # gather chunks
    tpc = chunk // 128                # token-tiles per chunk
    bcol = s_row // 16                # wrapped col stride of row-start tokens

    h_dt = mybir.dt.float16 if DTYPE == "f16" else mybir.dt.bfloat16
    f32 = mybir.dt.float32
    i16, i32 = mybir.dt.int16, mybir.dt.int32
    Alu = mybir.AluOpType

    nc = bacc.Bacc("TRN2", target_bir_lowering=False, debug=debug,
                   num_swdge_queues=NQ)
    with tile.TileContext(nc) as tc:
        with (
            tc.tile_pool(name="dram", bufs=1, space="DRAM") as dram,
            tc.tile_pool(name="const", bufs=1) as const,
            tc.tile_pool(name="htp", bufs=HT_BUFS) as htp,
            tc.tile_pool(name="psum", bufs=8, space="PSUM") as psp,
            tc.tile_pool(name="outp", bufs=3) as outp,
        ):
            # idx_a/idx_b are host-prewrapped int32: [16, cols] with
            # a[p, c] = ext[c*16 + p], b[p, c] = ext[c*16 + p + 1] where
            # ext = [0] + idx_flat. Pure layout transform on the host.
            idx_a = dram.tile([16, cols], i32, kind="ExternalInput", name="idx_a", uniquify=False)
            idx_b = dram.tile([16, cols], i32, kind="ExternalInput", name="idx_b", uniquify=False)
            table = dram.tile([BUCKETS, ELEM], h_dt, kind="ExternalInput", name="table_pad", uniquify=False)
            proj_hi = dram.tile([P, DIM], h_dt, kind="ExternalInput", name="proj_hi", uniquify=False)
            if HILO:
                proj_lo = dram.tile([P, DIM], h_dt, kind="ExternalInput", name="proj_lo", uniquify=False)
            out = dram.tile([ntok, DIM], f32, kind="ExternalOutput", name="out", uniquify=False)

            pj_hi = const.tile([P, DIM], h_dt)
            nc.sync.dma_start(pj_hi[:, :], proj_hi[:, :])
            if HILO:
                pj_lo = const.tile([P, DIM], h_dt)
                nc.sync.dma_start(pj_lo[:, :], proj_lo[:, :])

            # bigram = ((a & 8191) * 1815 + b) & 8191  (== (a*10007+b) % 8192)
            # Computed on partitions 0-15 (DVE partition base must be 0/32/..),
            # then replicated to all 128 partitions via SBUF->SBUF DMA
            # (gpsimd cores each read their own 16-row replica).
            ia = const.tile([16, cols], i32)
            ib = const.tile([16, cols], i32)
            nc.sync.dma_start(ia[:, :], idx_a[:, :])
            nc.sync.dma_start(ib[:, :], idx_b[:, :])
            tmp = const.tile([16, cols], i32)
            w16 = const.tile([P, cols], i16)
            nc.vector.tensor_scalar(tmp[:, :], ia[:, :], 8191, None, op0=Alu.bitwise_and)
            nc.vector.tensor_scalar(tmp[:, :], tmp[:, :], 1815, None, op0=Alu.mult)
            nc.vector.tensor_tensor(tmp[:, :], tmp[:, :], ib[:, :], op=Alu.add)
            nc.vector.tensor_scalar(tmp[:, :], tmp[:, :], 8191, None, op0=Alu.bitwise_and)
            # int32 -> int16: little-endian low half, stride-2 copy
            tmp16 = tmp.bitcast(i16).rearrange("p (c two) -> p c two", two=2)
            nc.vector.tensor_copy(w16[0:16, :], tmp16[:, :, 0])
            # Row-start tokens use bigram index 0 (left pad).
            w16v = w16.rearrange("p (r c) -> p r c", c=bcol)
            nc.vector.memset(w16v[0:1, :, 0], 0)
            for r in range(1, 8):
                nc.sync.dma_start(w16[16 * r:16 * r + 16, :], w16[0:16, :])

            # chunk schedule: ramp in with small chunks so the first matmuls
            # start as early as possible, then steady-state CHUNK-token gathers
            ramp = [r for r in (512, 512, 1024) if r < chunk]
            if ramp and sum(ramp) % chunk == 0 and ntok > sum(ramp):
                sched = ramp + [chunk] * ((ntok - sum(ramp)) // chunk)
            else:
                sched = [chunk] * (ntok // chunk)
            assert sum(sched) == ntok

            out_view = out.rearrange("(G j p) o -> G p j o", p=P, j=GRP)
            tok0 = 0          # running token offset
            gi = 0            # gather index (for queue rotation)
            for csz in sched:
                ht = htp.tile([P, 1, chunk], h_dt, name="ht", tag="ht")
                nc.gpsimd.dma_gather(
                    ht[:, 0:1, 0:csz],
                    table[:, :],
                    w16[:, tok0 // 16:(tok0 + csz) // 16],
                    csz,
                    csz,
                    ELEM,
                    transpose=True,
                    single_packet=SP,
                    queue_num=gi % NQ,
                )
                gi += 1
                for g in range(csz // (GRP * 128)):
                    ot = outp.tile([P, GRP, DIM], f32, name="ot", tag="ot")
                    for j in range(GRP):
                        t = g * GRP + j           # token-tile within chunk
                        ti = tok0 // 128 + t      # global token-tile index
                        ps = psp.tile([P, DIM], f32, name="ps", tag="ps")
                        lhsT = ht[0:D_BIGRAM, 0, t * 128:(t + 1) * 128]
                        if HILO:
                            nc.tensor.matmul(ps[:, :], lhsT, pj_hi[0:D_BIGRAM, :], start=True, stop=False)
                            nc.tensor.matmul(ps[:, :], lhsT, pj_lo[0:D_BIGRAM, :], start=False, stop=True)
                        else:
                            nc.tensor.matmul(ps[:, :], lhsT, pj_hi[0:D_BIGRAM, :], start=True, stop=True)
                        if ti % 3 == 2:
                            nc.scalar.copy(ot[:, j, :], ps[:, :])
                        else:
                            nc.vector.tensor_copy(ot[:, j, :], ps[:, :])
                    nc.sync.dma_start(out_view[tok0 // (GRP * 128) + g], ot[:, :, :])
                tok0 += csz
    nc.compile()
    return nc


def _get_nc():
    key = (NTOK, S, CHUNK, HILO, DTYPE, HT_BUFS, NQ, SP)
    if key not in _CACHE:
        _CACHE[key] = _build(NTOK, S, CHUNK)
    return _CACHE[key]


def _host_inputs(idx: np.ndarray, table: np.ndarray, proj_w: np.ndarray):
    """Build the per-core input maps (host-side shard + layout glue)."""
    npdt = _np_dt()
    idx = np.asarray(idx)
    table = np.asarray(table)
    proj = np.asarray(proj_w, dtype=np.float32)

    table_pad = np.zeros((BUCKETS, ELEM), npdt)
    table_pad[:, :D_BIGRAM] = table.astype(npdt)

    projT = proj.T.astype(np.float32)                    # [48, 512]
    hi = np.zeros((P, DIM), npdt)
    hi[:D_BIGRAM] = projT.astype(npdt)
    if HILO:
        lo = np.zeros((P, DIM), npdt)
        lo[:D_BIGRAM] = (projT - hi[:D_BIGRAM].astype(np.float32)).astype(npdt)

    in_maps = []
    for c in range(N_CORES):
        shard = np.ascontiguousarray(idx[c * ROWS_PER_CORE:(c + 1) * ROWS_PER_CORE]).reshape(-1).astype(np.int32)
        ext = np.empty(NTOK + 1, np.int32)
        ext[0] = 0
        ext[1:] = shard
        # wrapped layout: [16, cols], element (p, c) = ext[c*16 + p]
        m = {
            "idx_a": np.ascontiguousarray(ext[0:NTOK].reshape(-1, 16).T),
            "idx_b": np.ascontiguousarray(ext[1:NTOK + 1].reshape(-1, 16).T),
            "table_pad": table_pad,
            "proj_hi": hi,
        }
        if HILO:
            m["proj_lo"] = lo
        in_maps.append(m)
    return in_maps


def kernel(idx, table, proj_w, _trace=False, _trace_kwargs=None):
    from concourse.bass_utils import run_bass_kernel_spmd

    nc = _get_nc()
    in_maps = _host_inputs(idx, table, proj_w)
    res = run_bass_kernel_spmd(
        nc,
        in_maps,
        core_ids=list(range(N_CORES)),
        trace=_trace,
        **(_trace_kwargs or {}),
    )
    outs = [r["out"].reshape(ROWS_PER_CORE, S, DIM) for r in res.results]
    full = np.concatenate(outs, axis=0)
    if _trace:
        return full, res
    return full


# revision 14
# speedup vs baseline: 1.0024x; 1.0024x over previous
"""Trainium2 Bass kernel for nn_BigramHash (hashed-bigram embedding + projection).

Computation (per reference):
    bigram_idx = pad_left0((idx[:, :-1] * 10007 + idx[:, 1:]) % 8192)   # [B, S]
    h = table[bigram_idx]                                               # fp16 [B, S, 48]
    out = h.astype(f32) @ proj_w.T                                      # f32 [B, S, 512]

Strategy (8-core data parallel over batch, 4 rows = 16384 tokens per core):
  - hash on DVE in int32 using (a & 8191) * 1815 + b (mod-2^13-equivalent,
    products < 2^24 so exact on any ALU path), then & 8191, cast to int16.
  - dma_gather(transpose=True) from the 256B-padded table in DRAM lands
    h^T in SBUF: partitions = d_bigram, free dim = tokens.
  - PE: per 128-token tile, lhsT = h^T slice [48, 128] (stationary),
    rhs = proj^T [48, 512] (optionally fp16 hi+lo pair accumulated in one
    PSUM group) -> out tile [128 tokens, 512] f32.
  - PSUM -> SBUF copies split DVE/ACT, then 1 MiB contiguous DMAs to DRAM.
"""

import os
import sys

sys.path.insert(0, "/opt/trn_rl_repo")

import numpy as np

N_CORES = 8
B, S = 32, 4096
BUCKETS, D_BIGRAM, DIM = 8192, 48, 512
ROWS_PER_CORE = B // N_CORES          # 4
NTOK = ROWS_PER_CORE * S              # 16384 tokens per core
ELEM = 128                            # padded table row: 128 x 2B = 256 B
P = 128

# Tunables (env-overridable for A/B benchmarking)
CHUNK = int(os.environ.get("KBH_CHUNK", "2048"))      # tokens per dma_gather
GRP = 4                                               # 128-token tiles per output DMA
HILO = os.environ.get("KBH_HILO", "0") == "1"         # hi/lo split of proj (2 matmuls)
DTYPE = os.environ.get("KBH_DTYPE", "f16")            # f16 | bf16 for h and proj
HT_BUFS = int(os.environ.get("KBH_HT_BUFS", "8"))
NQ = int(os.environ.get("KBH_NQ", "1"))               # SWDGE queues for gathers
SP = os.environ.get("KBH_SP", "0") == "1"             # single_packet gathers
SRC = os.environ.get("KBH_SRC", "dram")               # gather source: dram | sbuf

_CACHE: dict = {}


def _np_dt():
    if DTYPE == "f16":
        return np.float16
    import ml_dtypes
    return ml_dtypes.bfloat16


def _build(ntok: int, s_row: int, chunk: int, debug: bool = False):
    """Build the per-core Bass module. ntok tokens, rows of s_row tokens."""
    import concourse.mybir as mybir
    import concourse.tile as tile
    from concourse import bacc

    assert ntok % chunk == 0 and chunk % 128 == 0 and s_row % 16 == 0
    cols = ntok // 16                 # wrapped idx columns
    ccols = chunk // 16               # wrapped idx columns per chunk
    nch = ntok // chunk               # gather chunks
    tpc = chunk // 128                # token-tiles per chunk
    bcol = s_row // 16                # wrapped col stride of row-start tokens

    h_dt = mybir.dt.float16 if DTYPE == "f16" else mybir.dt.bfloat16
    f32 = mybir.dt.float32
    i16, i32 = mybir.dt.int16, mybir.dt.int32
    Alu = mybir.AluOpType

    nc = bacc.Bacc("TRN2", target_bir_lowering=False, debug=debug,
                   num_swdge_queues=NQ)
    with tile.TileContext(nc) as tc:
        with (
            tc.tile_pool(name="dram", bufs=1, space="DRAM") as dram,
            tc.tile_pool(name="const", bufs=1) as const,
            tc.tile_pool(name="htp", bufs=HT_BUFS) as htp,
            tc.tile_pool(name="psum", bufs=8, space="PSUM") as psp,
            tc.tile_pool(name="outp", bufs=3) as outp,
        ):
            # idx_a/idx_b are host-prewrapped int32: [16, cols] with
            # a[p, c] = ext[c*16 + p], b[p, c] = ext[c*16 + p + 1] where
            # ext = [0] + idx_flat. Pure layout transform on the host.
            idx_a = dram.tile([16, cols], i32, kind="ExternalInput", name="idx_a", uniquify=False)
            idx_b = dram.tile([16, cols], i32, kind="ExternalInput", name="idx_b", uniquify=False)
            table = dram.tile([BUCKETS, ELEM], h_dt, kind="ExternalInput", name="table_pad", uniquify=False)
            proj_hi = dram.tile([P, DIM], h_dt, kind="ExternalInput", name="proj_hi", uniquify=False)
            if HILO:
                proj_lo = dram.tile([P, DIM], h_dt, kind="ExternalInput", name="proj_lo", uniquify=False)
            out = dram.tile([ntok, DIM], f32, kind="ExternalOutput", name="out", uniquify=False)

            pj_hi = const.tile([P, DIM], h_dt)
            nc.sync.dma_start(pj_hi[:, :], proj_hi[:, :])
            if HILO:
                pj_lo = const.tile([P, DIM], h_dt)
                nc.sync.dma_start(pj_lo[:, :], proj_lo[:, :])

            if SRC == "sbuf":
                # Table resident in SBUF for low-latency gather reads:
                # partition p, rank stripe r (256 B) = table row r*128 + p.
                table_sb = const.tile([P, BUCKETS // P * ELEM], h_dt)
                nc.sync.dma_start(
                    table_sb.rearrange("p (r e) -> p r e", e=ELEM),
                    table.rearrange("(r p) e -> p r e", p=P),
                )

            # bigram = ((a & 8191) * 1815 + b) & 8191  (== (a*10007+b) % 8192)
            # Computed on partitions 0-15 (DVE partition base must be 0/32/..),
            # then replicated to all 128 partitions via SBUF->SBUF DMA
            # (gpsimd cores each read their own 16-row replica).
            ia = const.tile([16, cols], i32)
            ib = const.tile([16, cols], i32)
            nc.sync.dma_start(ia[:, :], idx_a[:, :])
            nc.sync.dma_start(ib[:, :], idx_b[:, :])
            tmp = const.tile([16, cols], i32)
            w16 = const.tile([P, cols], i16)
            nc.vector.tensor_scalar(tmp[:, :], ia[:, :], 8191, None, op0=Alu.bitwise_and)
            nc.vector.tensor_scalar(tmp[:, :], tmp[:, :], 1815, None, op0=Alu.mult)
            nc.vector.tensor_tensor(tmp[:, :], tmp[:, :], ib[:, :], op=Alu.add)
            nc.vector.tensor_scalar(tmp[:, :], tmp[:, :], 8191, None, op0=Alu.bitwise_and)
            # int32 -> int16: little-endian low half, stride-2 copy
            tmp16 = tmp.bitcast(i16).rearrange("p (c two) -> p c two", two=2)
            nc.vector.tensor_copy(w16[0:16, :], tmp16[:, :, 0])
            # Row-start tokens use bigram index 0 (left pad).
            w16v = w16.rearrange("p (r c) -> p r c", c=bcol)
            nc.vector.memset(w16v[0:1, :, 0], 0)
            for r in range(1, 8):
                nc.sync.dma_start(w16[16 * r:16 * r + 16, :], w16[0:16, :])

            # chunk schedule: ramp in with small chunks so the first matmuls
            # start as early as possible, then steady-state CHUNK-token gathers
            ramp = [r for r in (512, 512, 1024) if r < chunk]
            if ramp and sum(ramp) % chunk == 0 and ntok > sum(ramp):
                sched = ramp + [chunk] * ((ntok - sum(ramp)) // chunk)
            else:
                sched = [chunk] * (ntok // chunk)
            assert sum(sched) == ntok

            out_view = out.rearrange("(G j p) o -> G p j o", p=P, j=GRP)
            tok0 = 0          # running token offset
            gi = 0            # gather index (for queue rotation)
            for csz in sched:
                ht = htp.tile([P, 1, chunk], h_dt, name="ht", tag="ht")
                if SRC == "sbuf":
                    nc.gpsimd.dma_gather(
                        ht[:, 0:1, 0:csz],
                        table_sb[:, :],
                        w16[:, tok0 // 16:(tok0 + csz) // 16],
                        csz,
                        csz,
                        ELEM,
                        transpose=True,
                        single_packet=SP,
                        queue_num=gi % NQ,
                        sbuf_tokens_per_rank=P,
                        sbuf_free_dim_per_rank=ELEM * 2,
                        sbuf_free_dim_pad_per_rank=0,
                        sbuf_byte_offset=0,
                    )
                else:
                    nc.gpsimd.dma_gather(
                        ht[:, 0:1, 0:csz],
                        table[:, :],
                        w16[:, tok0 // 16:(tok0 + csz) // 16],
                        csz,
                        csz,
                        ELEM,
                        transpose=True,
                        single_packet=SP,
                        queue_num=gi % NQ,
                    )
                gi += 1
                for g in range(csz // (GRP * 128)):
                    ot = outp.tile([P, GRP, DIM], f32, name="ot", tag="ot")
                    for j in range(GRP):
                        t = g * GRP + j           # token-tile within chunk
                        ti = tok0 // 128 + t      # global token-tile index
                        ps = psp.tile([P, DIM], f32, name="ps", tag="ps")
                        lhsT = ht[0:D_BIGRAM, 0, t * 128:(t + 1) * 128]
                        if HILO:
                            nc.tensor.matmul(ps[:, :], lhsT, pj_hi[0:D_BIGRAM, :], start=True, stop=False)
                            nc.tensor.matmul(ps[:, :], lhsT, pj_lo[0:D_BIGRAM, :], start=False, stop=True)
                        else:
                            nc.tensor.matmul(ps[:, :], lhsT, pj_hi[0:D_BIGRAM, :], start=True, stop=True)
                        if ti % 3 == 2:
                            nc.scalar.copy(ot[:, j, :], ps[:, :])
                        else:
                            nc.vector.tensor_copy(ot[:, j, :], ps[:, :])
                    nc.sync.dma_start(out_view[tok0 // (GRP * 128) + g], ot[:, :, :])
                tok0 += csz
    nc.compile()
    return nc


def _get_nc():
    key = (NTOK, S, CHUNK, HILO, DTYPE, HT_BUFS, NQ, SP, SRC)
    if key not in _CACHE:
        _CACHE[key] = _build(NTOK, S, CHUNK)
    return _CACHE[key]


def _host_inputs(idx: np.ndarray, table: np.ndarray, proj_w: np.ndarray):
    """Build the per-core input maps (host-side shard + layout glue)."""
    npdt = _np_dt()
    idx = np.asarray(idx)
    table = np.asarray(table)
    proj = np.asarray(proj_w, dtype=np.float32)

    table_pad = np.zeros((BUCKETS, ELEM), npdt)
    table_pad[:, :D_BIGRAM] = table.astype(npdt)

    projT = proj.T.astype(np.float32)                    # [48, 512]
    hi = np.zeros((P, DIM), npdt)
    hi[:D_BIGRAM] = projT.astype(npdt)
    if HILO:
        lo = np.zeros((P, DIM), npdt)
        lo[:D_BIGRAM] = (projT - hi[:D_BIGRAM].astype(np.float32)).astype(npdt)

    in_maps = []
    for c in range(N_CORES):
        shard = np.ascontiguousarray(idx[c * ROWS_PER_CORE:(c + 1) * ROWS_PER_CORE]).reshape(-1).astype(np.int32)
        ext = np.empty(NTOK + 1, np.int32)
        ext[0] = 0
        ext[1:] = shard
        # wrapped layout: [16, cols], element (p, c) = ext[c*16 + p]
        m = {
            "idx_a": np.ascontiguousarray(ext[0:NTOK].reshape(-1, 16).T),
            "idx_b": np.ascontiguousarray(ext[1:NTOK + 1].reshape(-1, 16).T),
            "table_pad": table_pad,
            "proj_hi": hi,
        }
        if HILO:
            m["proj_lo"] = lo
        in_maps.append(m)
    return in_maps


def kernel(idx, table, proj_w, _trace=False, _trace_kwargs=None):
    from concourse.bass_utils import run_bass_kernel_spmd

    nc = _get_nc()
    in_maps = _host_inputs(idx, table, proj_w)
    res = run_bass_kernel_spmd(
        nc,
        in_maps,
        core_ids=list(range(N_CORES)),
        trace=_trace,
        **(_trace_kwargs or {}),
    )
    outs = [r["out"].reshape(ROWS_PER_CORE, S, DIM) for r in res.results]
    full = np.concatenate(outs, axis=0)
    if _trace:
        return full, res
    return full
